# revision 50
# baseline (speedup 1.0000x reference)
"""
Trainium2 Bass kernel for Llama GQA decode attention (B=8, Q=4, H=4096,
32 Q-heads / 8 KV-heads, HD=128, S=4096 cached tokens, fp32).

Sharding: tensor-parallel over heads across 8 cores. Core c owns KV head c
and its 4 query heads: Wq/Wk/Wv column slices, Wo row slice, K/V cache
kv-head slice. Each core computes a partial [32, 4096] output (its heads'
contribution through Wo); the full output is the sum over cores (done on
host -- no collectives needed).

DMA-roofline oriented design (~20.4 MB/core: K fp8 4MB, V fp16 8.45MB,
Wqkv fp8 3MB, Wo fp16 4MB, io/consts ~1MB):
- All weights host-pre-tiled into SBUF layout so every DMA is a linear
  max-element-size transfer (4-8KB rows).
- ALL bulk transfers share the sync queue in consumption order: a single
  queue sustains ~415 GB/s; splitting across queues DEGRADES aggregate
  bandwidth. Order: hs8, wqkv, kv0..kv7 (kT then v per batch), wo x8,
  out. Only tiny const packs ride the scalar queue.
- Wq/Wk/Wv packed into ONE fp8 tensor; QKV projection runs in fp8
  DoubleRow perf mode (2 contraction tiles / 2x throughput per matmul).
- SCALE applied via the exp activation's scale arg (q/wq stay in fp8's
  normal range). fp16 phase-2 intermediates; rope rotate-half via a
  fp16 +-1 matrix matmul.
- Attention is software-pipelined at depth 2 (PE order scores_b,
  pV_{b-2}) with mod-3 buffer-tag rotation, so the per-batch exp
  latency is off the PE critical path; kT bufs=6 / v bufs=7 keep the
  DMA stream running without buffer-release stalls (the PE runs at MID
  clock for the first ~35us - time-based ramp - so early batches are
  2x slower and need the runway).
- New-token pV matmul contracts over all 32 token partitions of a
  zero-padded p-transpose (no per-batch SBUF->SBUF DMA); per-batch oT
  transpose matmuls hide inside the loop.
- Out-projection consumes the 8 Wo pieces right after the KV stream;
  fp16 output, 4 piecewise output DMAs.
"""

import os
import sys

sys.path.insert(0, "/opt/trn_rl_repo")

import numpy as np

import concourse.bass as bass  # noqa: F401
import concourse.tile as tile
from concourse import bacc, bass_utils, mybir

# Problem constants (hardcoded per contract)
B, Q, H = 8, 4, 4096
NH, NKV, HD = 32, 8, 128
G = NH // NKV            # 4 query heads per kv head
S = 4096                 # cache token capacity actually used
TOK = B * Q              # 32 total new tokens
GQ = G * Q               # 16 (head, query) pairs per batch
DC = G * HD              # 512 = per-core slice of the o/q head dim
N_CORES = 8
SCALE = 1.0 / (HD ** 0.5)
NEG = -1.0e30
KT = H // 128            # 32 contraction tiles over H
KP = KT // 2             # 16 DoubleRow pair-tiles

FP32 = mybir.dt.float32
FP16 = mybir.dt.float16
FP8 = mybir.dt.float8e4
Exp = mybir.ActivationFunctionType.Exp
DR = mybir.MatmulPerfMode.DoubleRow

# cpack32 column layout (fp32 consts: masks only)
C_MNEW = 0           # [0:16, 4] new-token causal mask
C_MBND = 4           # [128, 8*16] boundary masks, b-major
C_TOT = 132
# cpack16 column layout (fp16 consts)
H_RT = 0             # [128, 128] rotate-half matrix (lhsT layout)
H_ID = 128           # [0:16, 16] identity
H_COS = 144          # [128, 32] cos, cols (b q)
H_SIN = 176          # [128, 32]
H_TOT = 208


def _build_program(nts: tuple, rems: tuple):
    """Build + compile the Bass program, specialized on per-batch cached-tile
    counts `nts` (128-tiles) and boundary-tile valid-row counts `rems`."""
    nc = bacc.Bacc("TRN2", target_bir_lowering=False, debug=False,
                   num_devices=N_CORES)

    hs8_d = nc.dram_tensor("hs8", [128, KP, 2, TOK], FP8,
                           kind="ExternalInput").ap()
    wqkv_d = nc.dram_tensor("wqkv", [128, KP, 2, DC + 2 * HD], FP8,
                            kind="ExternalInput").ap()
    cpack_d = nc.dram_tensor("cpack", [128, C_TOT], FP32,
                             kind="ExternalInput").ap()
    cpk16_d = nc.dram_tensor("cpk16", [128, H_TOT], FP16,
                             kind="ExternalInput").ap()
    wo_d = nc.dram_tensor("wo", [128, G, 2, H // 2], FP16,
                          kind="ExternalInput").ap()
    kT_d = nc.dram_tensor("kT", [B, HD, S], FP8, kind="ExternalInput").ap()
    v_d = nc.dram_tensor("v", [B, 128, S // 128, HD + 1], FP16,
                         kind="ExternalInput").ap()
    out_d = nc.dram_tensor("out", [TOK, H], FP16, kind="ExternalOutput").ap()

    with tile.TileContext(nc) as tc:
        consts = tc.alloc_tile_pool(name="consts", bufs=1)
        wpool = tc.alloc_tile_pool(name="wtiles", bufs=1)
        kpool = tc.alloc_tile_pool(name="kp", bufs=6)
        vpool = tc.alloc_tile_pool(name="vp", bufs=7)
        ppool = tc.alloc_tile_pool(name="pbuf", bufs=1)
        work = tc.alloc_tile_pool(name="work", bufs=1)
        ps_a = tc.alloc_tile_pool(name="ps_a", bufs=1, space="PSUM")
        ps_sc = tc.alloc_tile_pool(name="ps_sc", bufs=2, space="PSUM")
        ps_o = tc.alloc_tile_pool(name="ps_o", bufs=1, space="PSUM")
        ps_f = tc.alloc_tile_pool(name="ps_f", bufs=2, space="PSUM")

        # ---- head DMAs -------------------------------------------------
        # sync queue: hs8 + the 4 wqkv chunks (QKV proj critical path)
        hs8_sb = consts.tile([128, KP, 2, TOK], FP8)
        nc.sync.dma_start(out=hs8_sb, in_=hs8_d)
        wqkv_sb = wpool.tile([128, KP, 2, DC + 2 * HD], FP8, tag="wqkv")
        WCH = 4  # pair-tiles per chunk
        for cidx in range(KP // WCH):
            sl = slice(cidx * WCH, (cidx + 1) * WCH)
            nc.sync.dma_start(out=wqkv_sb[:, sl], in_=wqkv_d[:, sl])
        # scalar queue: consts packs (needed by phase B / masks)
        cpack_sb = consts.tile([128, C_TOT], FP32)
        nc.scalar.dma_start(out=cpack_sb, in_=cpack_d)
        cpk16_sb = consts.tile([128, H_TOT], FP16)
        nc.scalar.dma_start(out=cpk16_sb, in_=cpk16_d)

        # All bulk streams share the sync queue: a single queue sustains the
        # full DMA bandwidth, splitting across queues degrades it.
        def issue_kv(b):
            if nts[b] > 0:
                kT_b = kpool.tile([128, S], FP8, tag="kT", name=f"kT{b}")
                nc.sync.dma_start(out=kT_b[:, :nts[b] * 128],
                                  in_=kT_d[b, :, :nts[b] * 128])
                v_b = vpool.tile([128, S // 128, HD + 1], FP16, tag="v",
                                 name=f"v{b}")
                nc.sync.dma_start(out=v_b[:, :nts[b], :],
                                  in_=v_d[b, :, :nts[b], :])
            else:
                kT_b = v_b = None
            kT_bufs.append(kT_b)
            v_bufs.append(v_b)

        kT_bufs = []
        v_bufs = []
        for b in range(min(5, B)):
            issue_kv(b)

        # wo tile (8 pieces drip-fed from the vector queue in the batch loop)
        wo_sb = wpool.tile([128, G, 2, H // 2], FP16, tag="wo")
        wo_pieces = [(g, h) for h in range(2) for g in range(G)]

        # ---- phase 1: QKV projection (fp8 DoubleRow) -------------------
        qn_ps = ps_a.tile([TOK, DC], FP32, tag="qn")     # [tok, (g, d)]
        kvn_ps = ps_sc.tile([TOK, 2 * HD], FP32, tag="sc")  # [tok, (k|v) d]
        for t in range(KP):
            st, sp = (t == 0), (t == KP - 1)
            nc.tensor.matmul(qn_ps, hs8_sb[:, t], wqkv_sb[:, t, :, 0:DC],
                             start=st, stop=sp, perf_mode=DR)
            nc.tensor.matmul(kvn_ps, hs8_sb[:, t],
                             wqkv_sb[:, t, :, DC:DC + 2 * HD],
                             start=st, stop=sp, perf_mode=DR)

        # ---- phase 2: transposes + RoPE (fp16 intermediates) -----------
        v_new = work.tile([TOK, HD + 1], FP16)
        nc.scalar.activation(v_new[:, 0:HD], kvn_ps[:, HD:2 * HD],
                             mybir.ActivationFunctionType.Copy)
        nc.vector.memset(v_new[:, HD:HD + 1], 1.0)

        qn_sb = work.tile([TOK, DC], FP16)
        nc.vector.tensor_copy(qn_sb, qn_ps)
        kn_sb = work.tile([TOK, HD], FP16)
        nc.scalar.activation(kn_sb, kvn_ps[:, 0:HD],
                             mybir.ActivationFunctionType.Copy)
        # DVE 32x32 block transposes + one merged scatter copy per c-block
        qbt_sb = work.tile([TOK, DC], FP16)    # blockwise-transposed q
        for g in range(G):
            nc.vector.transpose(qbt_sb[:, g * HD:(g + 1) * HD],
                                qn_sb[:, g * HD:(g + 1) * HD])
        kbt_sb = work.tile([TOK, HD], FP16)
        nc.vector.transpose(kbt_sb, kn_sb)

        qT0_sb = work.tile([128, B * GQ], FP16)   # [d, (b, g, qi)]
        qT0_g = qT0_sb.rearrange("p (bb g q) -> p g bb q", bb=B, g=G)
        qbt_v = qbt_sb.rearrange("n (g c bb q) -> n g c bb q", g=G, c=4,
                                 bb=B)
        for c in range(4):
            nc.vector.tensor_copy(qT0_g[c * 32:(c + 1) * 32],
                                  qbt_v[:, :, c, :, :])
        kT0_sb = work.tile([128, TOK], FP16)      # [d, tok]
        kbt_v = kbt_sb.rearrange("n (c i) -> n c i", c=4)
        for c in range(4):
            nc.gpsimd.tensor_copy(kT0_sb[c * 32:(c + 1) * 32, :],
                                  kbt_v[:, c, :])

        # RoPE: rotate-half via PE matmul against the constant matrix
        rt_ap = cpk16_sb[:, H_RT:H_RT + HD]
        qrot_ps = ps_a.tile([128, B * GQ], FP32, tag="qn")
        nc.tensor.matmul(qrot_ps, rt_ap, qT0_sb, start=True, stop=True)
        krot_ps = ps_sc.tile([128, TOK], FP32, tag="sc")
        nc.tensor.matmul(krot_ps, rt_ap, kT0_sb, start=True, stop=True)

        cosT = cpk16_sb[:, H_COS:H_COS + TOK]
        sinT = cpk16_sb[:, H_SIN:H_SIN + TOK]
        # cos/sin for qT0 layout: value depends on (d, b, qi); bcast over g
        cos_q = bass.AP(tensor=cosT.tensor, offset=cosT.offset,
                        ap=[cosT.ap[0], [Q, B], [0, G], [1, Q]])
        sin_q = bass.AP(tensor=sinT.tensor, offset=sinT.offset,
                        ap=[sinT.ap[0], [Q, B], [0, G], [1, Q]])
        qf_sb = work.tile([128, B, GQ], FP8)       # rope'd qT (unscaled)
        qf_gq = qf_sb.rearrange("p b (g q) -> p b g q", g=G)
        tmpq_sb = work.tile([128, B, G, Q], FP32)
        q3 = qT0_sb.rearrange("p (b g q) -> p b g q", b=B, g=G)
        qr3 = qrot_ps.rearrange("p (b g q) -> p b g q", b=B, g=G)
        nc.vector.tensor_mul(tmpq_sb, q3, cos_q)
        nc.vector.tensor_mul(qf_gq, qr3, sin_q)
        nc.vector.tensor_add(qf_gq, qf_gq, tmpq_sb)

        kf_sb = work.tile([128, TOK], FP8)        # rope'd kT
        tmpk_sb = work.tile([128, TOK], FP32)
        nc.gpsimd.tensor_mul(tmpk_sb, kT0_sb, cosT)
        nc.vector.tensor_mul(kf_sb, krot_ps, sinT)
        nc.gpsimd.tensor_add(kf_sb, kf_sb, tmpk_sb)

        qf_flat = qf_sb.rearrange("p b m -> p (b m)")
        mnew_ap = cpack_sb[0:GQ, C_MNEW:C_MNEW + Q]

        # ---- phase 3: attention, software-pipelined over batches -------
        # PE order is scores_b, pV_{b-1}, scores_{b+1}, pV_b, ... so the
        # exp of batch b overlaps PE work instead of stalling it.
        o_all_sb = work.tile([GQ, B, HD], FP16)   # o / denom, [gq, b, d]
        wo_next = 0
        # prefetch kv_{b+5} per iteration; all wo pieces enqueue right
        # after kv7 (same queue => transfers stay in this order)
        wo_plan = {3: 8}

        def scores_block(b):
            nt = nts[b]
            kT_b = kT_bufs[b]
            qf_b = qf_flat[:, b * GQ:(b + 1) * GQ]          # [128, 16]
            pT_sb = ppool.tile([128, max(nt, 1), GQ], FP16, tag=f"pT{b % 3}")
            hh = (max(nt, 1) + 1) // 2
            pT2_sb = ppool.tile([128, max(nt - hh, 1), GQ], FP16,
                                tag=f"pU{b % 3}")
            if nt > 0:
                scT_ps = ps_sc.tile([128, max(nt, 1) * GQ], FP32, tag="sc")
                for t in range(nt):
                    nc.tensor.matmul(scT_ps[:, t * GQ:(t + 1) * GQ],
                                     kT_b[:, t * 128:(t + 1) * 128], qf_b,
                                     start=(t == 0), stop=(t == nt - 1))
                if rems[b] < 128:  # mask invalid tail rows of last tile
                    mb_ap = cpack_sb[:, C_MBND + b * GQ:C_MBND + (b + 1) * GQ]
                    nc.vector.tensor_add(
                        scT_ps[:, (nt - 1) * GQ:nt * GQ],
                        scT_ps[:, (nt - 1) * GQ:nt * GQ], mb_ap)
                # split exp: pV's first matmuls gate on half 1 only
                nc.scalar.activation(
                    pT_sb.rearrange("p t m -> p (t m)")[:, :hh * GQ],
                    scT_ps[:, :hh * GQ], Exp, scale=float(SCALE))
                if nt > hh:
                    nc.scalar.activation(
                        pT2_sb.rearrange("p t m -> p (t m)")[:, :(nt - hh) * GQ],
                        scT_ps[:, hh * GQ:nt * GQ], Exp, scale=float(SCALE))

            # new-token scores [gq, jj] + causal triangle mask
            sn_ps = ps_o.tile([GQ, Q], FP32, tag=f"o{b % 3}")
            nc.tensor.matmul(sn_ps, qf_b, kf_sb[:, b * Q:(b + 1) * Q],
                             start=True, stop=True)
            nc.vector.tensor_add(sn_ps, sn_ps, mnew_ap)
            pn_sb = ppool.tile([TOK, TOK], FP16, tag=f"pn{b % 3}")
            nc.vector.memset(pn_sb, 0.0)
            nc.scalar.activation(pn_sb[:GQ, b * Q:(b + 1) * Q], sn_ps, Exp,
                                 scale=float(SCALE))
            pnt_sb = ppool.tile([TOK, TOK], FP16, tag=f"pnt{b % 3}")
            nc.vector.transpose(pnt_sb, pn_sb)
            return pT_sb, pT2_sb, pnt_sb

        def pv_block(b, pT_sb, pT2_sb, pnt_sb):
            nt = nts[b]
            hh = (max(nt, 1) + 1) // 2
            v_b = v_bufs[b]
            # o[gq, 0:128] accumulation; col 128 accumulates the softmax
            # denominator via V's ones column
            o_ps = ps_o.tile([GQ, HD + 1], FP32, tag=f"o{b % 3}")
            if nt > 0:
                for t in range(nt):
                    src_p = pT_sb[:, t, :] if t < hh else pT2_sb[:, t - hh, :]
                    nc.tensor.matmul(o_ps, src_p, v_b[:, t, :],
                                     start=(t == 0), stop=False)
            nc.tensor.matmul(o_ps, pnt_sb[:, :GQ], v_new,
                             start=(nt == 0), stop=True)
            rec_sb = ppool.tile([GQ, 1], FP32, tag=f"rec{b % 3}")
            nc.vector.reciprocal(rec_sb, o_ps[:, HD:HD + 1])
            nc.vector.tensor_scalar_mul(o_all_sb[:, b, :], o_ps[:, 0:HD],
                                        rec_sb)
            nc.tensor.matmul(oT_ps[:, b, :], o_all_sb[:, b, :], id16_ap,
                             start=True, stop=True)
            nc.vector.tensor_copy(
                oT_sb[:, :, b, :],
                oT_ps[:, b, :].rearrange("p (g q) -> p g q", g=G))

        # oT accumulates per batch right after each pv_block
        id16_ap = cpk16_sb[0:GQ, H_ID:H_ID + GQ]
        oT_ps = ps_a.tile([128, B, GQ], FP32, tag="qn")
        oT_sb = work.tile([128, G, B, Q], FP16)

        pend: dict = {}
        for b in range(B):
            nb = b + 5
            if nb < B:
                issue_kv(nb)
            if wo_next < len(wo_pieces):
                for _ in range(wo_plan.get(b, 0)):
                    if wo_next < len(wo_pieces):
                        g, h = wo_pieces[wo_next]
                        nc.sync.dma_start(out=wo_sb[:, g, h, :],
                                          in_=wo_d[:, g, h, :])
                        wo_next += 1
            pend[b] = scores_block(b)
            if b >= 2:
                pv_block(b - 2, *pend.pop(b - 2))
        pv_block(B - 2, *pend.pop(B - 2))
        pv_block(B - 1, *pend.pop(B - 1))

        # ---- phase 4: output projection, two H-halves ------------------
        out_sb = work.tile([TOK, H], FP16)
        oT_flat = oT_sb.rearrange("p g b q -> p (g b q)")
        for n in range(8):
            h, nn = n // 4, n % 4
            if n % 3 == 2:
                fo_ps = ps_a.tile([TOK, 512], FP32, tag="qn")
            else:
                fo_ps = ps_f.tile([TOK, 512], FP32, tag="fo")
            for g in range(G):
                nc.tensor.matmul(
                    fo_ps, oT_flat[:, g * TOK:(g + 1) * TOK],
                    wo_sb[:, g, h, nn * 512:(nn + 1) * 512],
                    start=(g == 0), stop=(g == G - 1))
            dst = out_sb[:, n * 512:(n + 1) * 512]
            if n % 2 == 0:
                nc.vector.tensor_copy(dst, fo_ps)
            else:
                nc.scalar.activation(
                    dst, fo_ps, mybir.ActivationFunctionType.Copy)
            nc.sync.dma_start(
                out=out_d[:, n * 512:(n + 1) * 512],
                in_=out_sb[:, n * 512:(n + 1) * 512])

        ps_f.release()
        ps_o.release()
        ps_sc.release()
        ps_a.release()
        work.release()
        ppool.release()
        vpool.release()
        kpool.release()
        wpool.release()
        consts.release()

    nc.compile()
    return nc


_PROGRAM_CACHE: dict = {}


def _get_program(nts, rems):
    key = (tuple(nts), tuple(rems))
    if key not in _PROGRAM_CACHE:
        _PROGRAM_CACHE[key] = _build_program(tuple(nts), tuple(rems))
    return _PROGRAM_CACHE[key]


def _prep_inputs(hidden_states, cos, sin, Wq, Wk, Wv, Wo, K_cache, V_cache,
                 cache_lens):
    """Host-side shard prep. Returns (in_maps, nts, rems)."""
    f32 = np.float32
    f16 = np.float16
    f8 = mybir.dt.np(FP8)
    hs = np.asarray(hidden_states, dtype=f32).reshape(TOK, H)
    # hs8[p, t, i, n] = hs[n, (2t+i)*128 + p]
    hs8 = np.ascontiguousarray(
        hs.reshape(TOK, KP, 2, 128).transpose(3, 1, 2, 0)).astype(f8)

    lens = np.asarray(cache_lens, dtype=np.int64)
    nts, rems = [], []
    for b in range(B):
        ln = int(min(max(lens[b], 0), S))
        nt = (ln + 127) // 128
        rem = ln - (nt - 1) * 128 if nt > 0 else 128
        nts.append(nt)
        rems.append(rem)

    # consts packs
    cpack = np.zeros((128, C_TOT), dtype=f32)
    # new-token causal triangle: query qi sees new position jj iff jj <= qi
    for g in range(G):
        for qi in range(Q):
            for jj in range(Q):
                if jj > qi:
                    cpack[g * Q + qi, C_MNEW + jj] = NEG
    # boundary masks: rows >= rem of a batch's last cached tile are invalid
    for b in range(B):
        if nts[b] > 0 and rems[b] < 128:
            cpack[rems[b]:, C_MBND + b * GQ:C_MBND + (b + 1) * GQ] = NEG

    cpk16 = np.zeros((128, H_TOT), dtype=f16)
    # rotate-half matrix (transposed for lhsT use):
    # rot[d'] = -q[d'+64] for d'<64 ; +q[d'-64] for d'>=64
    hh = HD // 2
    R = np.zeros((HD, HD), dtype=f32)
    for dp in range(hh):
        R[dp, dp + hh] = -1.0
        R[dp + hh, dp] = 1.0
    cpk16[:, H_RT:H_RT + HD] = R.T
    cpk16[0:GQ, H_ID:H_ID + GQ] = np.eye(GQ, dtype=f16)
    cpk16[:, H_COS:H_COS + TOK] = np.asarray(cos, f32).reshape(TOK, HD).T
    cpk16[:, H_SIN:H_SIN + TOK] = np.asarray(sin, f32).reshape(TOK, HD).T

    in_maps = []
    for c in range(N_CORES):
        # packed QKV weights [4096, 768] -> [128, KP, 2, 768] fp8 (unscaled)
        wqkv = np.concatenate(
            [np.asarray(Wq[:, c * DC:(c + 1) * DC], f32),
             np.asarray(Wk[:, c * HD:(c + 1) * HD], f32),
             np.asarray(Wv[:, c * HD:(c + 1) * HD], f32)], axis=1)
        wqkv8 = np.ascontiguousarray(
            wqkv.reshape(KP, 2, 128, DC + 2 * HD).transpose(2, 0, 1, 3)
        ).astype(f8)
        # wo16[p, g, h, j] = Wo[c*512 + g*128 + p, h*2048 + j]
        wo = np.ascontiguousarray(
            np.asarray(Wo[c * DC:(c + 1) * DC, :], f32)
            .reshape(G, 128, 2, H // 2).transpose(1, 0, 2, 3)).astype(f16)
        kT = np.ascontiguousarray(
            K_cache[:, :S, c, :].transpose(0, 2, 1)).astype(f8)
        # v tiled + ones column: v4[b, p, t, 0:128] = V[b, t*128+p, :],
        # v4[b, p, t, 128] = 1.0 (accumulates softmax denominators)
        v = np.empty((B, 128, S // 128, HD + 1), dtype=f16)
        v[..., 0:HD] = (np.asarray(V_cache[:, :S, c, :], dtype=f32)
                        .reshape(B, S // 128, 128, HD).transpose(0, 2, 1, 3))
        v[..., HD] = 1.0
        in_maps.append(dict(hs8=hs8, wqkv=wqkv8, cpack=cpack, cpk16=cpk16,
                            wo=wo, kT=kT, v=v))
    return in_maps, nts, rems


def _install_axon_ntff_hook():
    """The agent image's antenv lacks axon_hooks; recreate the NTFF profile
    hook via ctypes against libaxon_pjrt.so so trace=True yields exec times."""
    try:
        from antenv.axon_hooks import get_axon_ntff_profile_hook  # noqa: F401
        return
    except ImportError:
        pass
    import contextlib
    import ctypes
    import types

    so_path = "/opt/axon/libaxon_pjrt.so"
    try:
        lib = ctypes.CDLL(so_path)
    except OSError:
        return
    if not hasattr(lib, "axon_start_nrt_profile"):
        return
    lib.axon_start_nrt_profile.argtypes = [ctypes.POINTER(ctypes.c_int64),
                                           ctypes.c_size_t]
    lib.axon_start_nrt_profile.restype = ctypes.c_int64
    lib.axon_stop_nrt_profile.argtypes = [ctypes.c_char_p]
    lib.axon_stop_nrt_profile.restype = ctypes.c_int64

    @contextlib.contextmanager
    def _hook(output_dir, device_ids):
        import jax
        jax.devices()
        if device_ids:
            ids = (ctypes.c_int64 * len(device_ids))(*device_ids)
            rc = lib.axon_start_nrt_profile(ids, len(device_ids))
        else:
            rc = lib.axon_start_nrt_profile(None, 0)
        if rc != 0:
            raise RuntimeError(f"axon_start_nrt_profile rc={rc}")
        try:
            yield
        finally:
            n = lib.axon_stop_nrt_profile(str(output_dir).encode())
            if n <= 0:
                print(f"profile: rc={n} writing to {output_dir}",
                      file=sys.stderr)

    import antenv
    mod = types.ModuleType("antenv.axon_hooks")
    mod.get_axon_ntff_profile_hook = lambda: _hook
    mod.set_axon_ntff_profile_hook = lambda h: None
    sys.modules["antenv.axon_hooks"] = mod
    antenv.axon_hooks = mod


_LAST_RESULTS = {}


def kernel(hidden_states, cos, sin, Wq, Wk, Wv, Wo, K_cache, V_cache,
           cache_lens):
    in_maps, nts, rems = _prep_inputs(hidden_states, cos, sin, Wq, Wk, Wv,
                                      Wo, K_cache, V_cache, cache_lens)
    nc = _get_program(nts, rems)

    trace = bool(int(os.environ.get("BASS_KERNEL_TRACE", "0")))
    if trace:
        _install_axon_ntff_hook()
    res = bass_utils.run_bass_kernel_spmd(
        nc, in_maps, core_ids=list(range(N_CORES)), trace=trace)
    _LAST_RESULTS["res"] = res

    total = np.zeros((TOK, H), dtype=np.float64)
    for c in range(N_CORES):
        total += res.results[c]["out"].astype(np.float64)
    return total.astype(np.float32).reshape(B, Q, H)


# revision 51
# speedup vs baseline: 1.0022x; 1.0022x over previous
"""
Trainium2 Bass kernel for Llama GQA decode attention (B=8, Q=4, H=4096,
32 Q-heads / 8 KV-heads, HD=128, S=4096 cached tokens, fp32).

Sharding: tensor-parallel over heads across 8 cores. Core c owns KV head c
and its 4 query heads: Wq/Wk/Wv column slices, Wo row slice, K/V cache
kv-head slice. Each core computes a partial [32, 4096] output (its heads'
contribution through Wo); the full output is the sum over cores (done on
host -- no collectives needed).

DMA-roofline oriented design (~20.4 MB/core: K fp8 4MB, V fp16 8.45MB,
Wqkv fp8 3MB, Wo fp16 4MB, io/consts ~1MB):
- All weights host-pre-tiled into SBUF layout so every DMA is a linear
  max-element-size transfer (4-8KB rows).
- ALL bulk transfers share the sync queue in consumption order: a single
  queue sustains ~415 GB/s; splitting across queues DEGRADES aggregate
  bandwidth. Order: hs8, wqkv, kv0..kv7 (kT then v per batch), wo x8,
  out. Only tiny const packs ride the scalar queue.
- Wq/Wk/Wv packed into ONE fp8 tensor; QKV projection runs in fp8
  DoubleRow perf mode (2 contraction tiles / 2x throughput per matmul).
- SCALE applied via the exp activation's scale arg (q/wq stay in fp8's
  normal range). fp16 phase-2 intermediates; rope rotate-half via a
  fp16 +-1 matrix matmul.
- Attention is software-pipelined at depth 2 (PE order scores_b,
  pV_{b-2}) with mod-3 buffer-tag rotation, so the per-batch exp
  latency is off the PE critical path; kT bufs=6 / v bufs=7 keep the
  DMA stream running without buffer-release stalls (the PE runs at MID
  clock for the first ~35us - time-based ramp - so early batches are
  2x slower and need the runway).
- New-token pV matmul contracts over all 32 token partitions of a
  zero-padded p-transpose (no per-batch SBUF->SBUF DMA); per-batch oT
  transpose matmuls hide inside the loop.
- Out-projection consumes the 8 Wo pieces right after the KV stream;
  fp16 output, 4 piecewise output DMAs.
"""

import os
import sys

sys.path.insert(0, "/opt/trn_rl_repo")

import numpy as np

import concourse.bass as bass  # noqa: F401
import concourse.tile as tile
from concourse import bacc, bass_utils, mybir

# Problem constants (hardcoded per contract)
B, Q, H = 8, 4, 4096
NH, NKV, HD = 32, 8, 128
G = NH // NKV            # 4 query heads per kv head
S = 4096                 # cache token capacity actually used
TOK = B * Q              # 32 total new tokens
GQ = G * Q               # 16 (head, query) pairs per batch
DC = G * HD              # 512 = per-core slice of the o/q head dim
N_CORES = 8
SCALE = 1.0 / (HD ** 0.5)
NEG = -1.0e30
KT = H // 128            # 32 contraction tiles over H
KP = KT // 2             # 16 DoubleRow pair-tiles

FP32 = mybir.dt.float32
FP16 = mybir.dt.float16
FP8 = mybir.dt.float8e4
Exp = mybir.ActivationFunctionType.Exp
DR = mybir.MatmulPerfMode.DoubleRow

# cpack32 column layout (fp32 consts: masks only)
C_MNEW = 0           # [0:16, 4] new-token causal mask
C_MBND = 4           # [128, 8*16] boundary masks, b-major
C_TOT = 132
# cpack16 column layout (fp16 consts)
H_RT = 0             # [128, 128] rotate-half matrix (lhsT layout)
H_ID = 128           # [0:16, 16] identity
H_COS = 144          # [128, 32] cos, cols (b q)
H_SIN = 176          # [128, 32]
H_TOT = 208


def _build_program(nts: tuple, rems: tuple):
    """Build + compile the Bass program, specialized on per-batch cached-tile
    counts `nts` (128-tiles) and boundary-tile valid-row counts `rems`."""
    nc = bacc.Bacc("TRN2", target_bir_lowering=False, debug=False,
                   num_devices=N_CORES)

    hs8_d = nc.dram_tensor("hs8", [128, KP, 2, TOK], FP8,
                           kind="ExternalInput").ap()
    wqkv_d = nc.dram_tensor("wqkv", [128, KP, 2, DC + 2 * HD], FP8,
                            kind="ExternalInput").ap()
    cpack_d = nc.dram_tensor("cpack", [128, C_TOT], FP32,
                             kind="ExternalInput").ap()
    cpk16_d = nc.dram_tensor("cpk16", [128, H_TOT], FP16,
                             kind="ExternalInput").ap()
    wo_d = nc.dram_tensor("wo", [128, G, 2, H // 2], FP16,
                          kind="ExternalInput").ap()
    kT_d = nc.dram_tensor("kT", [B, HD, S], FP8, kind="ExternalInput").ap()
    v_d = nc.dram_tensor("v", [B, 128, S // 128, HD + 1], FP16,
                         kind="ExternalInput").ap()
    out_d = nc.dram_tensor("out", [TOK, H], FP16, kind="ExternalOutput").ap()

    with tile.TileContext(nc) as tc:
        consts = tc.alloc_tile_pool(name="consts", bufs=1)
        wpool = tc.alloc_tile_pool(name="wtiles", bufs=1)
        kpool = tc.alloc_tile_pool(name="kp", bufs=6)
        vpool = tc.alloc_tile_pool(name="vp", bufs=7)
        ppool = tc.alloc_tile_pool(name="pbuf", bufs=1)
        work = tc.alloc_tile_pool(name="work", bufs=1)
        ps_a = tc.alloc_tile_pool(name="ps_a", bufs=1, space="PSUM")
        ps_sc = tc.alloc_tile_pool(name="ps_sc", bufs=2, space="PSUM")
        ps_o = tc.alloc_tile_pool(name="ps_o", bufs=1, space="PSUM")
        ps_f = tc.alloc_tile_pool(name="ps_f", bufs=2, space="PSUM")

        # ---- head DMAs -------------------------------------------------
        # sync queue: hs8 + the 4 wqkv chunks (QKV proj critical path)
        hs8_sb = consts.tile([128, KP, 2, TOK], FP8)
        nc.sync.dma_start(out=hs8_sb, in_=hs8_d)
        wqkv_sb = wpool.tile([128, KP, 2, DC + 2 * HD], FP8, tag="wqkv")
        WCH = 4  # pair-tiles per chunk
        for cidx in range(KP // WCH):
            sl = slice(cidx * WCH, (cidx + 1) * WCH)
            nc.sync.dma_start(out=wqkv_sb[:, sl], in_=wqkv_d[:, sl])
        # scalar queue: consts packs (needed by phase B / masks)
        cpack_sb = consts.tile([128, C_TOT], FP32)
        nc.scalar.dma_start(out=cpack_sb, in_=cpack_d)
        cpk16_sb = consts.tile([128, H_TOT], FP16)
        nc.scalar.dma_start(out=cpk16_sb, in_=cpk16_d)

        # All bulk streams share the sync queue: a single queue sustains the
        # full DMA bandwidth, splitting across queues degrades it.
        def issue_kv(b):
            if nts[b] > 0:
                kT_b = kpool.tile([128, S], FP8, tag="kT", name=f"kT{b}")
                nc.sync.dma_start(out=kT_b[:, :nts[b] * 128],
                                  in_=kT_d[b, :, :nts[b] * 128])
                v_b = vpool.tile([128, S // 128, HD + 1], FP16, tag="v",
                                 name=f"v{b}")
                nc.sync.dma_start(out=v_b[:, :nts[b], :],
                                  in_=v_d[b, :, :nts[b], :])
            else:
                kT_b = v_b = None
            kT_bufs.append(kT_b)
            v_bufs.append(v_b)

        kT_bufs = []
        v_bufs = []
        for b in range(min(5, B)):
            issue_kv(b)

        # wo tile (8 pieces drip-fed from the vector queue in the batch loop)
        wo_sb = wpool.tile([128, G, 2, H // 2], FP16, tag="wo")
        wo_pieces = [(g, h) for h in range(2) for g in range(G)]

        # ---- phase 1: QKV projection (fp8 DoubleRow) -------------------
        qn_ps = ps_a.tile([TOK, DC], FP32, tag="qn")     # [tok, (g, d)]
        kvn_ps = ps_sc.tile([TOK, 2 * HD], FP32, tag="sc")  # [tok, (k|v) d]
        for t in range(KP):
            st, sp = (t == 0), (t == KP - 1)
            nc.tensor.matmul(qn_ps, hs8_sb[:, t], wqkv_sb[:, t, :, 0:DC],
                             start=st, stop=sp, perf_mode=DR)
            nc.tensor.matmul(kvn_ps, hs8_sb[:, t],
                             wqkv_sb[:, t, :, DC:DC + 2 * HD],
                             start=st, stop=sp, perf_mode=DR)

        # ---- phase 2: transposes + RoPE (fp16 intermediates) -----------
        v_new = work.tile([TOK, HD + 1], FP16)
        nc.scalar.activation(v_new[:, 0:HD], kvn_ps[:, HD:2 * HD],
                             mybir.ActivationFunctionType.Copy)
        nc.vector.memset(v_new[:, HD:HD + 1], 1.0)

        qn_sb = work.tile([TOK, DC], FP16)
        nc.vector.tensor_copy(qn_sb, qn_ps)
        kn_sb = work.tile([TOK, HD], FP16)
        nc.scalar.activation(kn_sb, kvn_ps[:, 0:HD],
                             mybir.ActivationFunctionType.Copy)
        # DVE 32x32 block transposes + one merged scatter copy per c-block
        qbt_sb = work.tile([TOK, DC], FP16)    # blockwise-transposed q
        for g in range(G):
            nc.vector.transpose(qbt_sb[:, g * HD:(g + 1) * HD],
                                qn_sb[:, g * HD:(g + 1) * HD])
        kbt_sb = work.tile([TOK, HD], FP16)
        nc.vector.transpose(kbt_sb, kn_sb)

        qT0_sb = work.tile([128, B * GQ], FP16)   # [d, (b, g, qi)]
        qT0_g = qT0_sb.rearrange("p (bb g q) -> p g bb q", bb=B, g=G)
        qbt_v = qbt_sb.rearrange("n (g c bb q) -> n g c bb q", g=G, c=4,
                                 bb=B)
        for c in range(4):
            nc.vector.tensor_copy(qT0_g[c * 32:(c + 1) * 32],
                                  qbt_v[:, :, c, :, :])
        kT0_sb = work.tile([128, TOK], FP16)      # [d, tok]
        kbt_v = kbt_sb.rearrange("n (c i) -> n c i", c=4)
        for c in range(4):
            nc.gpsimd.tensor_copy(kT0_sb[c * 32:(c + 1) * 32, :],
                                  kbt_v[:, c, :])

        # RoPE: rotate-half via PE matmul against the constant matrix
        rt_ap = cpk16_sb[:, H_RT:H_RT + HD]
        qrot_ps = ps_a.tile([128, B * GQ], FP32, tag="qn")
        nc.tensor.matmul(qrot_ps, rt_ap, qT0_sb, start=True, stop=True)
        krot_ps = ps_sc.tile([128, TOK], FP32, tag="sc")
        nc.tensor.matmul(krot_ps, rt_ap, kT0_sb, start=True, stop=True)

        cosT = cpk16_sb[:, H_COS:H_COS + TOK]
        sinT = cpk16_sb[:, H_SIN:H_SIN + TOK]
        # cos/sin for qT0 layout: value depends on (d, b, qi); bcast over g
        cos_q = bass.AP(tensor=cosT.tensor, offset=cosT.offset,
                        ap=[cosT.ap[0], [Q, B], [0, G], [1, Q]])
        sin_q = bass.AP(tensor=sinT.tensor, offset=sinT.offset,
                        ap=[sinT.ap[0], [Q, B], [0, G], [1, Q]])
        qf_sb = work.tile([128, B, GQ], FP8)       # rope'd qT (unscaled)
        qf_gq = qf_sb.rearrange("p b (g q) -> p b g q", g=G)
        tmpq_sb = work.tile([128, B, G, Q], FP32)
        q3 = qT0_sb.rearrange("p (b g q) -> p b g q", b=B, g=G)
        qr3 = qrot_ps.rearrange("p (b g q) -> p b g q", b=B, g=G)
        nc.vector.tensor_mul(tmpq_sb, q3, cos_q)
        nc.vector.tensor_mul(qf_gq, qr3, sin_q)
        nc.vector.tensor_add(qf_gq, qf_gq, tmpq_sb)

        kf_sb = work.tile([128, TOK], FP8)        # rope'd kT
        tmpk_sb = work.tile([128, TOK], FP32)
        nc.gpsimd.tensor_mul(tmpk_sb, kT0_sb, cosT)
        nc.vector.tensor_mul(kf_sb, krot_ps, sinT)
        nc.gpsimd.tensor_add(kf_sb, kf_sb, tmpk_sb)

        qf_flat = qf_sb.rearrange("p b m -> p (b m)")
        mnew_ap = cpack_sb[0:GQ, C_MNEW:C_MNEW + Q]

        # new-token scores/exp/transpose for ALL batches upfront: they only
        # need qf/kf, and doing them here removes their matmuls, exps and
        # WAR edges from the per-batch steady-state window
        pn_all = work.tile([TOK, B, TOK], FP16)
        pnt_all = work.tile([TOK, B, TOK], FP16)
        nc.vector.memset(pn_all, 0.0)
        for b in range(B):
            sn_ps = ps_o.tile([GQ, Q], FP32, tag=f"o{b % 3}")
            nc.tensor.matmul(sn_ps, qf_flat[:, b * GQ:(b + 1) * GQ],
                             kf_sb[:, b * Q:(b + 1) * Q],
                             start=True, stop=True)
            nc.vector.tensor_add(sn_ps, sn_ps, mnew_ap)
            nc.scalar.activation(pn_all[:GQ, b, b * Q:(b + 1) * Q], sn_ps,
                                 Exp, scale=float(SCALE))
            nc.vector.transpose(pnt_all[:, b, :], pn_all[:, b, :])

        # ---- phase 3: attention, software-pipelined over batches -------
        # PE order is scores_b, pV_{b-1}, scores_{b+1}, pV_b, ... so the
        # exp of batch b overlaps PE work instead of stalling it.
        o_all_sb = work.tile([GQ, B, HD], FP16)   # o / denom, [gq, b, d]
        wo_next = 0
        # prefetch kv_{b+5} per iteration; all wo pieces enqueue right
        # after kv7 (same queue => transfers stay in this order)
        wo_plan = {3: 8}

        def scores_block(b):
            nt = nts[b]
            kT_b = kT_bufs[b]
            qf_b = qf_flat[:, b * GQ:(b + 1) * GQ]          # [128, 16]
            pT_sb = ppool.tile([128, max(nt, 1), GQ], FP16, tag=f"pT{b % 3}")
            hh = (max(nt, 1) + 1) // 2
            pT2_sb = ppool.tile([128, max(nt - hh, 1), GQ], FP16,
                                tag=f"pU{b % 3}")
            if nt > 0:
                scT_ps = ps_sc.tile([128, max(nt, 1) * GQ], FP32, tag="sc")
                for t in range(nt):
                    nc.tensor.matmul(scT_ps[:, t * GQ:(t + 1) * GQ],
                                     kT_b[:, t * 128:(t + 1) * 128], qf_b,
                                     start=(t == 0), stop=(t == nt - 1))
                if rems[b] < 128:  # mask invalid tail rows of last tile
                    mb_ap = cpack_sb[:, C_MBND + b * GQ:C_MBND + (b + 1) * GQ]
                    nc.vector.tensor_add(
                        scT_ps[:, (nt - 1) * GQ:nt * GQ],
                        scT_ps[:, (nt - 1) * GQ:nt * GQ], mb_ap)
                # split exp: pV's first matmuls gate on half 1 only
                nc.scalar.activation(
                    pT_sb.rearrange("p t m -> p (t m)")[:, :hh * GQ],
                    scT_ps[:, :hh * GQ], Exp, scale=float(SCALE))
                if nt > hh:
                    nc.scalar.activation(
                        pT2_sb.rearrange("p t m -> p (t m)")[:, :(nt - hh) * GQ],
                        scT_ps[:, hh * GQ:nt * GQ], Exp, scale=float(SCALE))

            return pT_sb, pT2_sb

        def pv_block(b, pT_sb, pT2_sb):
            nt = nts[b]
            hh = (max(nt, 1) + 1) // 2
            v_b = v_bufs[b]
            # o[gq, 0:128] accumulation; col 128 accumulates the softmax
            # denominator via V's ones column
            o_ps = ps_o.tile([GQ, HD + 1], FP32, tag=f"o{b % 3}")
            if nt > 0:
                for t in range(nt):
                    src_p = pT_sb[:, t, :] if t < hh else pT2_sb[:, t - hh, :]
                    nc.tensor.matmul(o_ps, src_p, v_b[:, t, :],
                                     start=(t == 0), stop=False)
            nc.tensor.matmul(o_ps, pnt_all[:, b, :GQ], v_new,
                             start=(nt == 0), stop=True)
            rec_sb = ppool.tile([GQ, 1], FP32, tag=f"rec{b % 3}")
            nc.vector.reciprocal(rec_sb, o_ps[:, HD:HD + 1])
            nc.vector.tensor_scalar_mul(o_all_sb[:, b, :], o_ps[:, 0:HD],
                                        rec_sb)
            nc.tensor.matmul(oT_ps[:, b, :], o_all_sb[:, b, :], id16_ap,
                             start=True, stop=True)
            nc.vector.tensor_copy(
                oT_sb[:, :, b, :],
                oT_ps[:, b, :].rearrange("p (g q) -> p g q", g=G))

        # oT accumulates per batch right after each pv_block
        id16_ap = cpk16_sb[0:GQ, H_ID:H_ID + GQ]
        oT_ps = ps_a.tile([128, B, GQ], FP32, tag="qn")
        oT_sb = work.tile([128, G, B, Q], FP16)

        pend: dict = {}
        for b in range(B):
            nb = b + 5
            if nb < B:
                issue_kv(nb)
            if wo_next < len(wo_pieces):
                for _ in range(wo_plan.get(b, 0)):
                    if wo_next < len(wo_pieces):
                        g, h = wo_pieces[wo_next]
                        nc.sync.dma_start(out=wo_sb[:, g, h, :],
                                          in_=wo_d[:, g, h, :])
                        wo_next += 1
            pend[b] = scores_block(b)
            if b >= 2:
                pv_block(b - 2, *pend.pop(b - 2))
        pv_block(B - 2, *pend.pop(B - 2))
        pv_block(B - 1, *pend.pop(B - 1))

        # ---- phase 4: output projection, two H-halves ------------------
        out_sb = work.tile([TOK, H], FP16)
        oT_flat = oT_sb.rearrange("p g b q -> p (g b q)")
        for n in range(8):
            h, nn = n // 4, n % 4
            if n % 3 == 2:
                fo_ps = ps_a.tile([TOK, 512], FP32, tag="qn")
            else:
                fo_ps = ps_f.tile([TOK, 512], FP32, tag="fo")
            for g in range(G):
                nc.tensor.matmul(
                    fo_ps, oT_flat[:, g * TOK:(g + 1) * TOK],
                    wo_sb[:, g, h, nn * 512:(nn + 1) * 512],
                    start=(g == 0), stop=(g == G - 1))
            dst = out_sb[:, n * 512:(n + 1) * 512]
            if n % 2 == 0:
                nc.vector.tensor_copy(dst, fo_ps)
            else:
                nc.scalar.activation(
                    dst, fo_ps, mybir.ActivationFunctionType.Copy)
            nc.sync.dma_start(
                out=out_d[:, n * 512:(n + 1) * 512],
                in_=out_sb[:, n * 512:(n + 1) * 512])

        ps_f.release()
        ps_o.release()
        ps_sc.release()
        ps_a.release()
        work.release()
        ppool.release()
        vpool.release()
        kpool.release()
        wpool.release()
        consts.release()

    nc.compile()
    return nc


_PROGRAM_CACHE: dict = {}


def _get_program(nts, rems):
    key = (tuple(nts), tuple(rems))
    if key not in _PROGRAM_CACHE:
        _PROGRAM_CACHE[key] = _build_program(tuple(nts), tuple(rems))
    return _PROGRAM_CACHE[key]


def _prep_inputs(hidden_states, cos, sin, Wq, Wk, Wv, Wo, K_cache, V_cache,
                 cache_lens):
    """Host-side shard prep. Returns (in_maps, nts, rems)."""
    f32 = np.float32
    f16 = np.float16
    f8 = mybir.dt.np(FP8)
    hs = np.asarray(hidden_states, dtype=f32).reshape(TOK, H)
    # hs8[p, t, i, n] = hs[n, (2t+i)*128 + p]
    hs8 = np.ascontiguousarray(
        hs.reshape(TOK, KP, 2, 128).transpose(3, 1, 2, 0)).astype(f8)

    lens = np.asarray(cache_lens, dtype=np.int64)
    nts, rems = [], []
    for b in range(B):
        ln = int(min(max(lens[b], 0), S))
        nt = (ln + 127) // 128
        rem = ln - (nt - 1) * 128 if nt > 0 else 128
        nts.append(nt)
        rems.append(rem)

    # consts packs
    cpack = np.zeros((128, C_TOT), dtype=f32)
    # new-token causal triangle: query qi sees new position jj iff jj <= qi
    for g in range(G):
        for qi in range(Q):
            for jj in range(Q):
                if jj > qi:
                    cpack[g * Q + qi, C_MNEW + jj] = NEG
    # boundary masks: rows >= rem of a batch's last cached tile are invalid
    for b in range(B):
        if nts[b] > 0 and rems[b] < 128:
            cpack[rems[b]:, C_MBND + b * GQ:C_MBND + (b + 1) * GQ] = NEG

    cpk16 = np.zeros((128, H_TOT), dtype=f16)
    # rotate-half matrix (transposed for lhsT use):
    # rot[d'] = -q[d'+64] for d'<64 ; +q[d'-64] for d'>=64
    hh = HD // 2
    R = np.zeros((HD, HD), dtype=f32)
    for dp in range(hh):
        R[dp, dp + hh] = -1.0
        R[dp + hh, dp] = 1.0
    cpk16[:, H_RT:H_RT + HD] = R.T
    cpk16[0:GQ, H_ID:H_ID + GQ] = np.eye(GQ, dtype=f16)
    cpk16[:, H_COS:H_COS + TOK] = np.asarray(cos, f32).reshape(TOK, HD).T
    cpk16[:, H_SIN:H_SIN + TOK] = np.asarray(sin, f32).reshape(TOK, HD).T

    in_maps = []
    for c in range(N_CORES):
        # packed QKV weights [4096, 768] -> [128, KP, 2, 768] fp8 (unscaled)
        wqkv = np.concatenate(
            [np.asarray(Wq[:, c * DC:(c + 1) * DC], f32),
             np.asarray(Wk[:, c * HD:(c + 1) * HD], f32),
             np.asarray(Wv[:, c * HD:(c + 1) * HD], f32)], axis=1)
        wqkv8 = np.ascontiguousarray(
            wqkv.reshape(KP, 2, 128, DC + 2 * HD).transpose(2, 0, 1, 3)
        ).astype(f8)
        # wo16[p, g, h, j] = Wo[c*512 + g*128 + p, h*2048 + j]
        wo = np.ascontiguousarray(
            np.asarray(Wo[c * DC:(c + 1) * DC, :], f32)
            .reshape(G, 128, 2, H // 2).transpose(1, 0, 2, 3)).astype(f16)
        kT = np.ascontiguousarray(
            K_cache[:, :S, c, :].transpose(0, 2, 1)).astype(f8)
        # v tiled + ones column: v4[b, p, t, 0:128] = V[b, t*128+p, :],
        # v4[b, p, t, 128] = 1.0 (accumulates softmax denominators)
        v = np.empty((B, 128, S // 128, HD + 1), dtype=f16)
        v[..., 0:HD] = (np.asarray(V_cache[:, :S, c, :], dtype=f32)
                        .reshape(B, S // 128, 128, HD).transpose(0, 2, 1, 3))
        v[..., HD] = 1.0
        in_maps.append(dict(hs8=hs8, wqkv=wqkv8, cpack=cpack, cpk16=cpk16,
                            wo=wo, kT=kT, v=v))
    return in_maps, nts, rems


def _install_axon_ntff_hook():
    """The agent image's antenv lacks axon_hooks; recreate the NTFF profile
    hook via ctypes against libaxon_pjrt.so so trace=True yields exec times."""
    try:
        from antenv.axon_hooks import get_axon_ntff_profile_hook  # noqa: F401
        return
    except ImportError:
        pass
    import contextlib
    import ctypes
    import types

    so_path = "/opt/axon/libaxon_pjrt.so"
    try:
        lib = ctypes.CDLL(so_path)
    except OSError:
        return
    if not hasattr(lib, "axon_start_nrt_profile"):
        return
    lib.axon_start_nrt_profile.argtypes = [ctypes.POINTER(ctypes.c_int64),
                                           ctypes.c_size_t]
    lib.axon_start_nrt_profile.restype = ctypes.c_int64
    lib.axon_stop_nrt_profile.argtypes = [ctypes.c_char_p]
    lib.axon_stop_nrt_profile.restype = ctypes.c_int64

    @contextlib.contextmanager
    def _hook(output_dir, device_ids):
        import jax
        jax.devices()
        if device_ids:
            ids = (ctypes.c_int64 * len(device_ids))(*device_ids)
            rc = lib.axon_start_nrt_profile(ids, len(device_ids))
        else:
            rc = lib.axon_start_nrt_profile(None, 0)
        if rc != 0:
            raise RuntimeError(f"axon_start_nrt_profile rc={rc}")
        try:
            yield
        finally:
            n = lib.axon_stop_nrt_profile(str(output_dir).encode())
            if n <= 0:
                print(f"profile: rc={n} writing to {output_dir}",
                      file=sys.stderr)

    import antenv
    mod = types.ModuleType("antenv.axon_hooks")
    mod.get_axon_ntff_profile_hook = lambda: _hook
    mod.set_axon_ntff_profile_hook = lambda h: None
    sys.modules["antenv.axon_hooks"] = mod
    antenv.axon_hooks = mod


_LAST_RESULTS = {}


def kernel(hidden_states, cos, sin, Wq, Wk, Wv, Wo, K_cache, V_cache,
           cache_lens):
    in_maps, nts, rems = _prep_inputs(hidden_states, cos, sin, Wq, Wk, Wv,
                                      Wo, K_cache, V_cache, cache_lens)
    nc = _get_program(nts, rems)

    trace = bool(int(os.environ.get("BASS_KERNEL_TRACE", "0")))
    if trace:
        _install_axon_ntff_hook()
    res = bass_utils.run_bass_kernel_spmd(
        nc, in_maps, core_ids=list(range(N_CORES)), trace=trace)
    _LAST_RESULTS["res"] = res

    total = np.zeros((TOK, H), dtype=np.float64)
    for c in range(N_CORES):
        total += res.results[c]["out"].astype(np.float64)
    return total.astype(np.float32).reshape(B, Q, H)


# revision 52
# speedup vs baseline: 1.0057x; 1.0034x over previous
"""
Trainium2 Bass kernel for Llama GQA decode attention (B=8, Q=4, H=4096,
32 Q-heads / 8 KV-heads, HD=128, S=4096 cached tokens, fp32).

Sharding: tensor-parallel over heads across 8 cores. Core c owns KV head c
and its 4 query heads: Wq/Wk/Wv column slices, Wo row slice, K/V cache
kv-head slice. Each core computes a partial [32, 4096] output (its heads'
contribution through Wo); the full output is the sum over cores (done on
host -- no collectives needed).

DMA-roofline oriented design (~20.4 MB/core: K fp8 4MB, V fp16 8.45MB,
Wqkv fp8 3MB, Wo fp16 4MB, io/consts ~1MB):
- All weights host-pre-tiled into SBUF layout so every DMA is a linear
  max-element-size transfer (4-8KB rows).
- ALL bulk transfers share the sync queue in consumption order: a single
  queue sustains ~415 GB/s; splitting across queues DEGRADES aggregate
  bandwidth. Order: hs8, wqkv, kv0..kv7 (kT then v per batch), wo x8,
  out. Only tiny const packs ride the scalar queue.
- Wq/Wk/Wv packed into ONE fp8 tensor; QKV projection runs in fp8
  DoubleRow perf mode (2 contraction tiles / 2x throughput per matmul).
- SCALE applied via the exp activation's scale arg (q/wq stay in fp8's
  normal range). fp16 phase-2 intermediates; rope rotate-half via a
  fp16 +-1 matrix matmul.
- Attention is software-pipelined at depth 2 (PE order scores_b,
  pV_{b-2}) with mod-3 buffer-tag rotation, so the per-batch exp
  latency is off the PE critical path; kT bufs=6 / v bufs=7 keep the
  DMA stream running without buffer-release stalls (the PE runs at MID
  clock for the first ~35us - time-based ramp - so early batches are
  2x slower and need the runway).
- New-token pV matmul contracts over all 32 token partitions of a
  zero-padded p-transpose (no per-batch SBUF->SBUF DMA); per-batch oT
  transpose matmuls hide inside the loop.
- Out-projection consumes the 8 Wo pieces right after the KV stream;
  fp16 output, 4 piecewise output DMAs.
"""

import os
import sys

sys.path.insert(0, "/opt/trn_rl_repo")

import numpy as np

import concourse.bass as bass  # noqa: F401
import concourse.tile as tile
from concourse import bacc, bass_utils, mybir

# Problem constants (hardcoded per contract)
B, Q, H = 8, 4, 4096
NH, NKV, HD = 32, 8, 128
G = NH // NKV            # 4 query heads per kv head
S = 4096                 # cache token capacity actually used
TOK = B * Q              # 32 total new tokens
GQ = G * Q               # 16 (head, query) pairs per batch
DC = G * HD              # 512 = per-core slice of the o/q head dim
N_CORES = 8
SCALE = 1.0 / (HD ** 0.5)
NEG = -1.0e30
KT = H // 128            # 32 contraction tiles over H
KP = KT // 2             # 16 DoubleRow pair-tiles

FP32 = mybir.dt.float32
FP16 = mybir.dt.float16
FP8 = mybir.dt.float8e4
Exp = mybir.ActivationFunctionType.Exp
DR = mybir.MatmulPerfMode.DoubleRow

# cpack32 column layout (fp32 consts: masks only)
C_MNEW = 0           # [0:16, 4] new-token causal mask
C_MBND = 4           # [128, 8*16] boundary masks, b-major
C_TOT = 132
# cpack16 column layout (fp16 consts)
H_RT = 0             # [128, 128] rotate-half matrix (lhsT layout)
H_ID = 128           # [0:16, 16] identity
H_COS = 144          # [128, 32] cos, cols (b q)
H_SIN = 176          # [128, 32]
H_TOT = 208


def _build_program(nts: tuple, rems: tuple):
    """Build + compile the Bass program, specialized on per-batch cached-tile
    counts `nts` (128-tiles) and boundary-tile valid-row counts `rems`."""
    nc = bacc.Bacc("TRN2", target_bir_lowering=False, debug=False,
                   num_devices=N_CORES)

    hs8_d = nc.dram_tensor("hs8", [128, KP, 2, TOK], FP8,
                           kind="ExternalInput").ap()
    wqkv_d = nc.dram_tensor("wqkv", [128, KP, 2, DC + 2 * HD], FP8,
                            kind="ExternalInput").ap()
    cpack_d = nc.dram_tensor("cpack", [128, C_TOT], FP32,
                             kind="ExternalInput").ap()
    cpk16_d = nc.dram_tensor("cpk16", [128, H_TOT], FP16,
                             kind="ExternalInput").ap()
    wo_d = nc.dram_tensor("wo", [128, G, 2, H // 2], FP16,
                          kind="ExternalInput").ap()
    kT_d = nc.dram_tensor("kT", [B, HD, S], FP8, kind="ExternalInput").ap()
    v_d = nc.dram_tensor("v", [B, 128, S // 128, HD + 1], FP16,
                         kind="ExternalInput").ap()
    out_d = nc.dram_tensor("out", [TOK, H], FP16, kind="ExternalOutput").ap()

    with tile.TileContext(nc) as tc:
        consts = tc.alloc_tile_pool(name="consts", bufs=1)
        wpool = tc.alloc_tile_pool(name="wtiles", bufs=1)
        kpool = tc.alloc_tile_pool(name="kp", bufs=6)
        vpool = tc.alloc_tile_pool(name="vp", bufs=7)
        ppool = tc.alloc_tile_pool(name="pbuf", bufs=1)
        work = tc.alloc_tile_pool(name="work", bufs=1)
        ps_a = tc.alloc_tile_pool(name="ps_a", bufs=1, space="PSUM")
        ps_sc = tc.alloc_tile_pool(name="ps_sc", bufs=2, space="PSUM")
        ps_o = tc.alloc_tile_pool(name="ps_o", bufs=1, space="PSUM")
        ps_f = tc.alloc_tile_pool(name="ps_f", bufs=2, space="PSUM")

        # ---- head DMAs -------------------------------------------------
        # sync queue: hs8 + the 4 wqkv chunks (QKV proj critical path)
        hs8_sb = consts.tile([128, KP, 2, TOK], FP8)
        nc.sync.dma_start(out=hs8_sb, in_=hs8_d)
        wqkv_sb = wpool.tile([128, KP, 2, DC + 2 * HD], FP8, tag="wqkv")
        WCH = 4  # pair-tiles per chunk
        for cidx in range(KP // WCH):
            sl = slice(cidx * WCH, (cidx + 1) * WCH)
            nc.sync.dma_start(out=wqkv_sb[:, sl], in_=wqkv_d[:, sl])
        # scalar queue: consts packs (needed by phase B / masks)
        cpack_sb = consts.tile([128, C_TOT], FP32)
        nc.scalar.dma_start(out=cpack_sb, in_=cpack_d)
        cpk16_sb = consts.tile([128, H_TOT], FP16)
        nc.scalar.dma_start(out=cpk16_sb, in_=cpk16_d)

        # All bulk streams share the sync queue: a single queue sustains the
        # full DMA bandwidth, splitting across queues degrades it.
        def issue_kv(b):
            if nts[b] > 0:
                kT_b = kpool.tile([128, S], FP8, tag="kT", name=f"kT{b}")
                nc.sync.dma_start(out=kT_b[:, :nts[b] * 128],
                                  in_=kT_d[b, :, :nts[b] * 128])
                v_b = vpool.tile([128, S // 128, HD + 1], FP16, tag="v",
                                 name=f"v{b}")
                nc.sync.dma_start(out=v_b[:, :nts[b], :],
                                  in_=v_d[b, :, :nts[b], :])
            else:
                kT_b = v_b = None
            kT_bufs.append(kT_b)
            v_bufs.append(v_b)

        kT_bufs = []
        v_bufs = []
        for b in range(min(5, B)):
            issue_kv(b)

        # wo tile (8 pieces drip-fed from the vector queue in the batch loop)
        wo_sb = wpool.tile([128, G, 2, H // 2], FP16, tag="wo")
        wo_pieces = [(g, h) for h in range(2) for g in range(G)]

        # ---- phase 1: QKV projection (fp8 DoubleRow) -------------------
        qn_ps = ps_a.tile([TOK, DC], FP32, tag="qn")     # [tok, (g, d)]
        kvn_ps = ps_sc.tile([TOK, 2 * HD], FP32, tag="sc")  # [tok, (k|v) d]
        for t in range(KP):
            st, sp = (t == 0), (t == KP - 1)
            nc.tensor.matmul(qn_ps, hs8_sb[:, t], wqkv_sb[:, t, :, 0:DC],
                             start=st, stop=sp, perf_mode=DR)
            nc.tensor.matmul(kvn_ps, hs8_sb[:, t],
                             wqkv_sb[:, t, :, DC:DC + 2 * HD],
                             start=st, stop=sp, perf_mode=DR)

        # ---- phase 2: transposes + RoPE (fp16 intermediates) -----------
        v_new = work.tile([TOK, HD + 1], FP16)
        nc.scalar.activation(v_new[:, 0:HD], kvn_ps[:, HD:2 * HD],
                             mybir.ActivationFunctionType.Copy)
        nc.vector.memset(v_new[:, HD:HD + 1], 1.0)

        qn_sb = work.tile([TOK, DC], FP16)
        nc.vector.tensor_copy(qn_sb, qn_ps)
        kn_sb = work.tile([TOK, HD], FP16)
        nc.scalar.activation(kn_sb, kvn_ps[:, 0:HD],
                             mybir.ActivationFunctionType.Copy)
        # DVE 32x32 block transposes + one merged scatter copy per c-block
        qbt_sb = work.tile([TOK, DC], FP16)    # blockwise-transposed q
        for g in range(G):
            nc.vector.transpose(qbt_sb[:, g * HD:(g + 1) * HD],
                                qn_sb[:, g * HD:(g + 1) * HD])
        kbt_sb = work.tile([TOK, HD], FP16)
        nc.vector.transpose(kbt_sb, kn_sb)

        qT0_sb = work.tile([128, B * GQ], FP16)   # [d, (b, g, qi)]
        qT0_g = qT0_sb.rearrange("p (bb g q) -> p g bb q", bb=B, g=G)
        qbt_v = qbt_sb.rearrange("n (g c bb q) -> n g c bb q", g=G, c=4,
                                 bb=B)
        for c in range(4):
            nc.vector.tensor_copy(qT0_g[c * 32:(c + 1) * 32],
                                  qbt_v[:, :, c, :, :])
        kT0_sb = work.tile([128, TOK], FP16)      # [d, tok]
        kbt_v = kbt_sb.rearrange("n (c i) -> n c i", c=4)
        for c in range(4):
            nc.gpsimd.tensor_copy(kT0_sb[c * 32:(c + 1) * 32, :],
                                  kbt_v[:, c, :])

        # RoPE: rotate-half via PE matmul against the constant matrix
        rt_ap = cpk16_sb[:, H_RT:H_RT + HD]
        qrot_ps = ps_a.tile([128, B * GQ], FP32, tag="qn")
        nc.tensor.matmul(qrot_ps, rt_ap, qT0_sb, start=True, stop=True)
        krot_ps = ps_sc.tile([128, TOK], FP32, tag="sc")
        nc.tensor.matmul(krot_ps, rt_ap, kT0_sb, start=True, stop=True)

        cosT = cpk16_sb[:, H_COS:H_COS + TOK]
        sinT = cpk16_sb[:, H_SIN:H_SIN + TOK]
        # cos/sin for qT0 layout: value depends on (d, b, qi); bcast over g
        cos_q = bass.AP(tensor=cosT.tensor, offset=cosT.offset,
                        ap=[cosT.ap[0], [Q, B], [0, G], [1, Q]])
        sin_q = bass.AP(tensor=sinT.tensor, offset=sinT.offset,
                        ap=[sinT.ap[0], [Q, B], [0, G], [1, Q]])
        qf_sb = work.tile([128, B, GQ], FP8)       # rope'd qT (unscaled)
        qf_gq = qf_sb.rearrange("p b (g q) -> p b g q", g=G)
        tmpq_sb = work.tile([128, B, G, Q], FP32)
        q3 = qT0_sb.rearrange("p (b g q) -> p b g q", b=B, g=G)
        qr3 = qrot_ps.rearrange("p (b g q) -> p b g q", b=B, g=G)
        nc.vector.tensor_mul(tmpq_sb, q3, cos_q)
        nc.vector.tensor_mul(qf_gq, qr3, sin_q)
        nc.vector.tensor_add(qf_gq, qf_gq, tmpq_sb)

        kf_sb = work.tile([128, TOK], FP8)        # rope'd kT
        tmpk_sb = work.tile([128, TOK], FP32)
        nc.gpsimd.tensor_mul(tmpk_sb, kT0_sb, cosT)
        nc.vector.tensor_mul(kf_sb, krot_ps, sinT)
        nc.gpsimd.tensor_add(kf_sb, kf_sb, tmpk_sb)

        qf_flat = qf_sb.rearrange("p b m -> p (b m)")
        mnew_ap = cpack_sb[0:GQ, C_MNEW:C_MNEW + Q]

        # ---- phase 3: attention, software-pipelined over batches -------
        # PE order is scores_b, pV_{b-1}, scores_{b+1}, pV_b, ... so the
        # exp of batch b overlaps PE work instead of stalling it.
        o_all_sb = work.tile([GQ, B, HD], FP16)   # o / denom, [gq, b, d]
        wo_next = 0
        # prefetch kv_{b+5} per iteration; all wo pieces enqueue right
        # after kv7 (same queue => transfers stay in this order)
        wo_plan = {3: 8}

        def scores_block(b):
            nt = nts[b]
            kT_b = kT_bufs[b]
            qf_b = qf_flat[:, b * GQ:(b + 1) * GQ]          # [128, 16]
            pT_sb = ppool.tile([128, max(nt, 1), GQ], FP16, tag=f"pT{b % 3}")
            hh = (max(nt, 1) + 1) // 2
            pT2_sb = ppool.tile([128, max(nt - hh, 1), GQ], FP16,
                                tag=f"pU{b % 3}")
            if nt > 0:
                scT_ps = ps_sc.tile([128, max(nt, 1) * GQ], FP32, tag="sc")
                for t in range(nt):
                    nc.tensor.matmul(scT_ps[:, t * GQ:(t + 1) * GQ],
                                     kT_b[:, t * 128:(t + 1) * 128], qf_b,
                                     start=(t == 0), stop=(t == nt - 1))
                if rems[b] < 128:  # mask invalid tail rows of last tile
                    mb_ap = cpack_sb[:, C_MBND + b * GQ:C_MBND + (b + 1) * GQ]
                    nc.vector.tensor_add(
                        scT_ps[:, (nt - 1) * GQ:nt * GQ],
                        scT_ps[:, (nt - 1) * GQ:nt * GQ], mb_ap)
                # split exp: pV's first matmuls gate on half 1 only
                nc.scalar.activation(
                    pT_sb.rearrange("p t m -> p (t m)")[:, :hh * GQ],
                    scT_ps[:, :hh * GQ], Exp, scale=float(SCALE))
                if nt > hh:
                    nc.scalar.activation(
                        pT2_sb.rearrange("p t m -> p (t m)")[:, :(nt - hh) * GQ],
                        scT_ps[:, hh * GQ:nt * GQ], Exp, scale=float(SCALE))

            # new-token scores [gq, jj] + causal triangle mask
            sn_ps = ps_o.tile([GQ, Q], FP32, tag=f"o{b % 3}")
            nc.tensor.matmul(sn_ps, qf_b, kf_sb[:, b * Q:(b + 1) * Q],
                             start=True, stop=True)
            nc.vector.tensor_add(sn_ps, sn_ps, mnew_ap)
            pn_sb = ppool.tile([TOK, TOK], FP16, tag=f"pn{b % 3}")
            nc.vector.memset(pn_sb, 0.0)
            nc.scalar.activation(pn_sb[:GQ, b * Q:(b + 1) * Q], sn_ps, Exp,
                                 scale=float(SCALE))
            pnt_sb = ppool.tile([TOK, TOK], FP16, tag=f"pnt{b % 3}")
            nc.vector.transpose(pnt_sb, pn_sb)
            return pT_sb, pT2_sb, pnt_sb

        def pv_block(b, pT_sb, pT2_sb, pnt_sb):
            nt = nts[b]
            hh = (max(nt, 1) + 1) // 2
            v_b = v_bufs[b]
            # o[gq, 0:128] accumulation; col 128 accumulates the softmax
            # denominator via V's ones column
            o_ps = ps_o.tile([GQ, HD + 1], FP32, tag=f"o{b % 3}")
            if nt > 0:
                for t in range(nt):
                    src_p = pT_sb[:, t, :] if t < hh else pT2_sb[:, t - hh, :]
                    nc.tensor.matmul(o_ps, src_p, v_b[:, t, :],
                                     start=(t == 0), stop=False)
            nc.tensor.matmul(o_ps, pnt_sb[:, :GQ], v_new,
                             start=(nt == 0), stop=True)
            rec_sb = ppool.tile([GQ, 1], FP32, tag=f"rec{b % 3}")
            nc.vector.reciprocal(rec_sb, o_ps[:, HD:HD + 1])
            nc.vector.tensor_scalar_mul(o_all_sb[:, b, :], o_ps[:, 0:HD],
                                        rec_sb)
            nc.tensor.matmul(oT_ps[:, b, :], o_all_sb[:, b, :], id16_ap,
                             start=True, stop=True)
            nc.vector.tensor_copy(
                oT_sb[:, :, b, :],
                oT_ps[:, b, :].rearrange("p (g q) -> p g q", g=G))

        # oT accumulates per batch right after each pv_block
        id16_ap = cpk16_sb[0:GQ, H_ID:H_ID + GQ]
        oT_ps = ps_a.tile([128, B, GQ], FP32, tag="qn")
        oT_sb = work.tile([128, G, B, Q], FP16)

        pend: dict = {}
        for b in range(B):
            nb = b + 5
            if nb < B:
                issue_kv(nb)
            if wo_next < len(wo_pieces):
                for _ in range(wo_plan.get(b, 0)):
                    if wo_next < len(wo_pieces):
                        g, h = wo_pieces[wo_next]
                        nc.sync.dma_start(out=wo_sb[:, g, h, :],
                                          in_=wo_d[:, g, h, :])
                        wo_next += 1
            pend[b] = scores_block(b)
            if b >= 2:
                pv_block(b - 2, *pend.pop(b - 2))
        pv_block(B - 2, *pend.pop(B - 2))
        pv_block(B - 1, *pend.pop(B - 1))

        # ---- phase 4: output projection, two H-halves ------------------
        out_sb = work.tile([TOK, H], FP16)
        oT_flat = oT_sb.rearrange("p g b q -> p (g b q)")
        for n in range(8):
            h, nn = n // 4, n % 4
            if n % 3 == 2:
                fo_ps = ps_a.tile([TOK, 512], FP32, tag="qn")
            else:
                fo_ps = ps_f.tile([TOK, 512], FP32, tag="fo")
            for g in range(G):
                nc.tensor.matmul(
                    fo_ps, oT_flat[:, g * TOK:(g + 1) * TOK],
                    wo_sb[:, g, h, nn * 512:(nn + 1) * 512],
                    start=(g == 0), stop=(g == G - 1))
            dst = out_sb[:, n * 512:(n + 1) * 512]
            if n % 2 == 0:
                nc.vector.tensor_copy(dst, fo_ps)
            else:
                nc.scalar.activation(
                    dst, fo_ps, mybir.ActivationFunctionType.Copy)
            nc.sync.dma_start(
                out=out_d[:, n * 512:(n + 1) * 512],
                in_=out_sb[:, n * 512:(n + 1) * 512])

        ps_f.release()
        ps_o.release()
        ps_sc.release()
        ps_a.release()
        work.release()
        ppool.release()
        vpool.release()
        kpool.release()
        wpool.release()
        consts.release()

    nc.compile()
    return nc


_PROGRAM_CACHE: dict = {}


def _get_program(nts, rems):
    key = (tuple(nts), tuple(rems))
    if key not in _PROGRAM_CACHE:
        _PROGRAM_CACHE[key] = _build_program(tuple(nts), tuple(rems))
    return _PROGRAM_CACHE[key]


def _prep_inputs(hidden_states, cos, sin, Wq, Wk, Wv, Wo, K_cache, V_cache,
                 cache_lens):
    """Host-side shard prep. Returns (in_maps, nts, rems)."""
    f32 = np.float32
    f16 = np.float16
    f8 = mybir.dt.np(FP8)
    hs = np.asarray(hidden_states, dtype=f32).reshape(TOK, H)
    # hs8[p, t, i, n] = hs[n, (2t+i)*128 + p]
    hs8 = np.ascontiguousarray(
        hs.reshape(TOK, KP, 2, 128).transpose(3, 1, 2, 0)).astype(f8)

    lens = np.asarray(cache_lens, dtype=np.int64)
    nts, rems = [], []
    for b in range(B):
        ln = int(min(max(lens[b], 0), S))
        nt = (ln + 127) // 128
        rem = ln - (nt - 1) * 128 if nt > 0 else 128
        nts.append(nt)
        rems.append(rem)

    # consts packs
    cpack = np.zeros((128, C_TOT), dtype=f32)
    # new-token causal triangle: query qi sees new position jj iff jj <= qi
    for g in range(G):
        for qi in range(Q):
            for jj in range(Q):
                if jj > qi:
                    cpack[g * Q + qi, C_MNEW + jj] = NEG
    # boundary masks: rows >= rem of a batch's last cached tile are invalid
    for b in range(B):
        if nts[b] > 0 and rems[b] < 128:
            cpack[rems[b]:, C_MBND + b * GQ:C_MBND + (b + 1) * GQ] = NEG

    cpk16 = np.zeros((128, H_TOT), dtype=f16)
    # rotate-half matrix (transposed for lhsT use):
    # rot[d'] = -q[d'+64] for d'<64 ; +q[d'-64] for d'>=64
    hh = HD // 2
    R = np.zeros((HD, HD), dtype=f32)
    for dp in range(hh):
        R[dp, dp + hh] = -1.0
        R[dp + hh, dp] = 1.0
    cpk16[:, H_RT:H_RT + HD] = R.T
    cpk16[0:GQ, H_ID:H_ID + GQ] = np.eye(GQ, dtype=f16)
    cpk16[:, H_COS:H_COS + TOK] = np.asarray(cos, f32).reshape(TOK, HD).T
    cpk16[:, H_SIN:H_SIN + TOK] = np.asarray(sin, f32).reshape(TOK, HD).T

    in_maps = []
    for c in range(N_CORES):
        # packed QKV weights [4096, 768] -> [128, KP, 2, 768] fp8 (unscaled)
        wqkv = np.concatenate(
            [np.asarray(Wq[:, c * DC:(c + 1) * DC], f32),
             np.asarray(Wk[:, c * HD:(c + 1) * HD], f32),
             np.asarray(Wv[:, c * HD:(c + 1) * HD], f32)], axis=1)
        wqkv8 = np.ascontiguousarray(
            wqkv.reshape(KP, 2, 128, DC + 2 * HD).transpose(2, 0, 1, 3)
        ).astype(f8)
        # wo16[p, g, h, j] = Wo[c*512 + g*128 + p, h*2048 + j]
        wo = np.ascontiguousarray(
            np.asarray(Wo[c * DC:(c + 1) * DC, :], f32)
            .reshape(G, 128, 2, H // 2).transpose(1, 0, 2, 3)).astype(f16)
        kT = np.ascontiguousarray(
            K_cache[:, :S, c, :].transpose(0, 2, 1)).astype(f8)
        # v tiled + ones column: v4[b, p, t, 0:128] = V[b, t*128+p, :],
        # v4[b, p, t, 128] = 1.0 (accumulates softmax denominators)
        v = np.empty((B, 128, S // 128, HD + 1), dtype=f16)
        v[..., 0:HD] = (np.asarray(V_cache[:, :S, c, :], dtype=f32)
                        .reshape(B, S // 128, 128, HD).transpose(0, 2, 1, 3))
        v[..., HD] = 1.0
        in_maps.append(dict(hs8=hs8, wqkv=wqkv8, cpack=cpack, cpk16=cpk16,
                            wo=wo, kT=kT, v=v))
    return in_maps, nts, rems


def _install_axon_ntff_hook():
    """The agent image's antenv lacks axon_hooks; recreate the NTFF profile
    hook via ctypes against libaxon_pjrt.so so trace=True yields exec times."""
    try:
        from antenv.axon_hooks import get_axon_ntff_profile_hook  # noqa: F401
        return
    except ImportError:
        pass
    import contextlib
    import ctypes
    import types

    so_path = "/opt/axon/libaxon_pjrt.so"
    try:
        lib = ctypes.CDLL(so_path)
    except OSError:
        return
    if not hasattr(lib, "axon_start_nrt_profile"):
        return
    lib.axon_start_nrt_profile.argtypes = [ctypes.POINTER(ctypes.c_int64),
                                           ctypes.c_size_t]
    lib.axon_start_nrt_profile.restype = ctypes.c_int64
    lib.axon_stop_nrt_profile.argtypes = [ctypes.c_char_p]
    lib.axon_stop_nrt_profile.restype = ctypes.c_int64

    @contextlib.contextmanager
    def _hook(output_dir, device_ids):
        import jax
        jax.devices()
        if device_ids:
            ids = (ctypes.c_int64 * len(device_ids))(*device_ids)
            rc = lib.axon_start_nrt_profile(ids, len(device_ids))
        else:
            rc = lib.axon_start_nrt_profile(None, 0)
        if rc != 0:
            raise RuntimeError(f"axon_start_nrt_profile rc={rc}")
        try:
            yield
        finally:
            n = lib.axon_stop_nrt_profile(str(output_dir).encode())
            if n <= 0:
                print(f"profile: rc={n} writing to {output_dir}",
                      file=sys.stderr)

    import antenv
    mod = types.ModuleType("antenv.axon_hooks")
    mod.get_axon_ntff_profile_hook = lambda: _hook
    mod.set_axon_ntff_profile_hook = lambda h: None
    sys.modules["antenv.axon_hooks"] = mod
    antenv.axon_hooks = mod


_LAST_RESULTS = {}


def kernel(hidden_states, cos, sin, Wq, Wk, Wv, Wo, K_cache, V_cache,
           cache_lens):
    in_maps, nts, rems = _prep_inputs(hidden_states, cos, sin, Wq, Wk, Wv,
                                      Wo, K_cache, V_cache, cache_lens)
    nc = _get_program(nts, rems)

    trace = bool(int(os.environ.get("BASS_KERNEL_TRACE", "0")))
    if trace:
        _install_axon_ntff_hook()
    res = bass_utils.run_bass_kernel_spmd(
        nc, in_maps, core_ids=list(range(N_CORES)), trace=trace)
    _LAST_RESULTS["res"] = res

    total = np.zeros((TOK, H), dtype=np.float64)
    for c in range(N_CORES):
        total += res.results[c]["out"].astype(np.float64)
    return total.astype(np.float32).reshape(B, Q, H)
